# revision 31
# baseline (speedup 1.0000x reference)
"""Trainium2 Bass kernel for nn_Attn_82832739270904 (sparse dilated attention).

Math (per batch, L=1024, D=256):
  3 layers of dilated causal attention with window 2: layer i (d = 1,2,4)
  lets position t attend only to {t, t-d}.  So softmax rows have <=2
  nonzeros; attn output = two diagonals per [L,L] slice, rest exact 0.0.
    s0[t] = out[t].out[t]/16, s1[t] = out[t].out[t-d]/16
    (a0,a1) = softmax(s0,s1);  out' = (1+a0)*out + a1*out[t-d]
  Final: o = out3[:,16:] @ W^T + b, reshaped [.,4,64], + inp[:,16:,:64].

Sharding: pure data parallel, 2 batches per core across 8 cores.

Layout on chip: sequence on partitions (8 chunks of 128), D on free dim.
Shifted-by-d copies made via SBUF->SBUF DMA.  Dots via fused
tensor_tensor_reduce (DVE) / activation-Square-accumulate (ACT).  Apply via
scalar_tensor_tensor + activation-scale.  Final linear on PE after
PE-transposes, bias folded in as K=1 matmuls (bf16 hi/lo split), residual
added with a broadcast AP.  attn written as 24 x 1MiB zero DMAs + strided
diagonal DMAs.
"""

import os
import sys

import numpy as np

for _p in ("/opt/trn_rl_repo", "/root/.axon_site/_ro/trn_rl_repo"):
    if os.path.isdir(_p) and _p not in sys.path:
        sys.path.insert(0, _p)

import concourse.bacc as bacc
import concourse.bass as bass
import concourse.mybir as mybir
from concourse.masks import make_identity
from concourse.tile import TileContext

P = 128
B, L, DIN = 16, 1024, 256
PAST, FUT, DIM = 16, 4, 64
NL = 3
DILS = (1, 2, 4)
NCORES = 8
BPC = B // NCORES            # batches per core
NCH = L // P                 # seq chunks per batch
CW = DIN                     # cols per chunk in the [128, 2048] layout
ROW = NCH * CW
SLICE = L * L                # elements in one [L, L] attn slice
f32 = mybir.dt.float32
f32r = mybir.dt.float32r
bf16 = mybir.dt.bfloat16
AL = mybir.AluOpType
AF = mybir.ActivationFunctionType


def _dview(flat_ap, offset, dims):
    """Arbitrary strided view of a flat DRAM AP (for diagonal writes)."""
    v = flat_ap.copy()
    v.offset = offset
    v.ap = mybir.VecI64Pair(dims)
    return v


def build():
    nc = bacc.Bacc(None, target_bir_lowering=False, debug=False)
    inp = nc.dram_tensor("inp", [BPC, L, DIN], f32, kind="ExternalInput")
    wt = nc.dram_tensor("wt", [DIN, DIN], f32, kind="ExternalInput")  # W_out.T
    bhi = nc.dram_tensor("bhi", [1, DIN], bf16, kind="ExternalInput")
    blo = nc.dram_tensor("blo", [1, DIN], bf16, kind="ExternalInput")
    attn = nc.dram_tensor("attn", [BPC, NL, L, L], f32, kind="ExternalOutput")
    o = nc.dram_tensor("o", [BPC, L - PAST, FUT, DIM], f32, kind="ExternalOutput")

    attn_flat = attn[:].flatten()
    o_v = o[:].rearrange("b l f d -> b l (f d)")

    with TileContext(nc) as tc:
        with (
            tc.tile_pool(name="const", bufs=1) as cp,
            tc.tile_pool(name="outs", bufs=(NL + 1) * BPC) as outp,
            tc.tile_pool(name="shift", bufs=3) as shp,
            tc.tile_pool(name="scores", bufs=NL * BPC) as scp,
            tc.tile_pool(name="attw", bufs=NL * BPC) as awp,
            tc.tile_pool(name="soft", bufs=4) as sfp,
            tc.tile_pool(name="coef", bufs=NL * BPC) as cfp,
            tc.tile_pool(name="jdve", bufs=2) as jdp,
            tc.tile_pool(name="jact", bufs=2) as jap,
            tc.tile_pool(name="uu", bufs=3) as upool,
            tc.tile_pool(name="o3t", bufs=4) as tpool,
            tc.tile_pool(name="osb", bufs=3) as osp,
            tc.tile_pool(name="tps", bufs=4, space="PSUM") as tps,
            tc.tile_pool(name="ops", bufs=3, space="PSUM") as opsum,
        ):
            # ---- constants ----
            zeros = cp.tile([P, 2048], f32)
            nc.vector.memset(zeros[:], 0.0)
            ident = cp.tile([P, P], f32)
            make_identity(nc, ident[:])
            ones_bf = cp.tile([1, P], bf16)
            nc.vector.memset(ones_bf[:], 1.0)
            wt_t = []
            for h in range(2):
                t = cp.tile([P, DIN], f32r, name=f"wt{h}", tag=f"wt{h}")
                nc.gpsimd.dma_start(t[:], wt[h * P:(h + 1) * P, :])
                wt_t.append(t)
            bhi_t = cp.tile([1, DIN], bf16)
            nc.sync.dma_start(bhi_t[:], bhi[:])
            blo_t = cp.tile([1, DIN], bf16)
            nc.sync.dma_start(blo_t[:], blo[:])

            # ---- load inp into layer-0 out tiles ----
            out_t = [[outp.tile([P, ROW], f32, name=f"out_l{l}_b{b}", tag="out")
                      for b in range(BPC)] for l in range(NL + 1)]
            for b in range(BPC):
                nc.sync.dma_start(
                    out_t[0][b][:].rearrange("p (c d) -> p c d", c=NCH),
                    inp[b, :, :].rearrange("(c p) d -> p c d", p=P),
                )

            # ---- zero-fill attn (24 x 1MiB) on SWDGE: HWDGE traffic starves
            # SWDGE queues, so the bulk stream must be the SWDGE one ----
            for l in range(NL):
                for b in range(BPC):
                    base = (b * NL + l) * SLICE
                    for q in range(4):
                        dst = _dview(attn_flat, base + q * (SLICE // 4),
                                     [[2048, P], [1, 2048]])
                        nc.gpsimd.dma_start(dst, zeros[:])

            # ---- attention layers ----
            deferred_diags = []

            def emit_diags(l, b, d, a_t):
                base = (b * NL + l) * SLICE
                nc.gpsimd.dma_start(
                    _dview(attn_flat, base, [[1025, P], [P * 1025, NCH]]),
                    a_t[:, 0:NCH])
                nc.gpsimd.dma_start(
                    _dview(attn_flat, base + 1024 * d,
                           [[1025, P - d], [1, 1]]),
                    a_t[d:P, NCH:NCH + 1])
                nc.gpsimd.dma_start(
                    _dview(attn_flat, base + P * 1025 - d,
                           [[1025, P], [P * 1025, NCH - 1]]),
                    a_t[:, NCH + 1:2 * NCH])

            for l in range(NL):
                d = DILS[l]
                cur = out_t[l]
                nxt = out_t[l + 1]
                # shifts for both batches first so the SWDGE queue stays warm
                sh = []
                for b in range(BPC):
                    sht = shp.tile([P, ROW], f32, name=f"sh_{l}_{b}", tag="sht")
                    # keep chunk-0 rows < d finite (g there is 0)
                    nc.vector.memset(sht[0:4, 0:CW], 0.0)
                    nc.sync.dma_start(sht[d:P, :], cur[b][0:P - d, :])
                    nc.sync.dma_start(sht[0:d, CW:ROW], cur[b][P - d:P, 0:ROW - CW])
                    sh.append(sht)
                # previous layer's diagonal writes go out now: their zero-fill
                # and coefficients are both ready, so they don't stall the ring
                for args in deferred_diags:
                    emit_diags(*args)
                deferred_diags = []

                for b in range(BPC):
                    # per-batch coefficient tiles so the two batch pipelines
                    # never serialize on shared softmax state
                    s_t = scp.tile([P, 16], f32, name=f"s_{l}_{b}", tag="s_t")
                    a_t = awp.tile([P, 16], f32, name=f"a_{l}_{b}", tag="a_t")
                    m_t = sfp.tile([P, NCH], f32, name=f"m_{l}_{b}", tag="m_t")
                    z_t = sfp.tile([P, NCH], f32, name=f"z_{l}_{b}", tag="z_t")
                    r_t = sfp.tile([P, NCH], f32, name=f"r_{l}_{b}", tag="r_t")
                    p1_t = cfp.tile([P, NCH], f32, name=f"p1_{l}_{b}", tag="p1_t")
                    rg_t = sfp.tile([P, NCH], f32, name=f"rg_{l}_{b}", tag="rg_t")
                    g_t = cfp.tile([P, NCH], f32, name=f"g_{l}_{b}", tag="g_t")
                    sht = sh[b]

                    # dots: s0 on ACT (square+accum); s1 fused STT on DVE
                    for c in range(NCH):
                        cs = c * CW
                        ja = jap.tile([P, CW], f32)
                        nc.scalar.activation(
                            ja[:], cur[b][:, cs:cs + CW], AF.Square,
                            scale=0.25, accum_out=s_t[:, c:c + 1])
                        jv = jdp.tile([P, CW], f32)
                        eng = nc.vector
                        eng.scalar_tensor_tensor(
                            out=jv[:],
                            in0=cur[b][:, cs:cs + CW],
                            scalar=0.0625,
                            in1=sht[:, cs:cs + CW],
                            op0=AL.mult, op1=AL.mult,
                            accum_out=s_t[:, NCH + c:NCH + c + 1])

                    # rows t < d attend only to themselves: force s1 = -1e9
                    nc.vector.memset(s_t[0:d, NCH:NCH + 1], -1e9)

                    # pairwise softmax over (s0, s1) for this batch
                    nc.vector.tensor_tensor(m_t[:], s_t[:, 0:NCH],
                                            s_t[:, NCH:2 * NCH], AL.max)
                    sv = s_t[:].rearrange("p (x c) -> p x c", x=2)
                    mb = m_t[:].unsqueeze(1).broadcast_to([P, 2, NCH])
                    nc.vector.tensor_tensor(sv, sv, mb, AL.subtract)
                    nc.scalar.activation(a_t[:], s_t[:], AF.Exp)
                    nc.vector.tensor_tensor(z_t[:], a_t[:, 0:NCH],
                                            a_t[:, NCH:2 * NCH], AL.add)
                    nc.vector.reciprocal(r_t[:], z_t[:])
                    av = a_t[:].rearrange("p (x c) -> p x c", x=2)
                    rb = r_t[:].unsqueeze(1).broadcast_to([P, 2, NCH])
                    nc.vector.tensor_tensor(av, av, rb, AL.mult)
                    nc.vector.tensor_scalar_add(p1_t[:], a_t[:, 0:NCH], 1.0)
                    nc.vector.reciprocal(rg_t[:], p1_t[:])
                    nc.vector.tensor_tensor(g_t[:], a_t[:, NCH:2 * NCH],
                                            rg_t[:], AL.mult)

                    # diagonal writes deferred until the next layer's shifts
                    deferred_diags.append((l, b, d, a_t))

                    # apply: out' = (1+a0) * (out + g * shifted)
                    for c in range(NCH):
                        cs = c * CW
                        u = upool.tile([P, CW], f32)
                        nc.vector.scalar_tensor_tensor(
                            out=u[:],
                            in0=sht[:, cs:cs + CW],
                            scalar=g_t[:, c:c + 1],
                            in1=cur[b][:, cs:cs + CW],
                            op0=AL.mult, op1=AL.add)
                        if c % 2 == 0:
                            nc.scalar.activation(
                                nxt[b][:, cs:cs + CW], u[:], AF.Copy,
                                scale=p1_t[:, c:c + 1])
                        else:
                            nc.vector.tensor_scalar_mul(
                                nxt[b][:, cs:cs + CW], u[:], p1_t[:, c:c + 1])

            # last layer's diagonal writes
            for args in deferred_diags:
                emit_diags(*args)
            deferred_diags = []

            # ---- final linear + bias + residual ----
            fin = out_t[NL]
            for b in range(BPC):
                o3 = [tpool.tile([P, NCH * P], f32r, name=f"o3_b{b}_h{h}", tag="o3")
                      for h in range(2)]
                for c in range(NCH):
                    for h in range(2):
                        ps = tps.tile([P, P], f32)
                        nc.tensor.transpose(
                            ps[:], fin[b][:, c * CW + h * P:c * CW + (h + 1) * P],
                            ident[:])
                        nc.scalar.copy(o3[h][:, c * P:(c + 1) * P], ps[:])
                for c in range(NCH):
                    ps = opsum.tile([P, DIN], f32)
                    nc.tensor.matmul(ps[:], o3[0][:, c * P:(c + 1) * P], wt_t[0][:],
                                     start=True, stop=False)
                    nc.tensor.matmul(ps[:], o3[1][:, c * P:(c + 1) * P], wt_t[1][:],
                                     start=False, stop=False)
                    nc.tensor.matmul(ps[:], ones_bf[:], bhi_t[:],
                                     start=False, stop=False)
                    nc.tensor.matmul(ps[:], ones_bf[:], blo_t[:],
                                     start=False, stop=True)
                    osb = osp.tile([P, DIN], f32)
                    res = out_t[0][b][:, c * CW:c * CW + DIM]
                    resb = res.unsqueeze(1).broadcast_to([P, FUT, DIM])
                    nc.vector.scalar_tensor_tensor(
                        out=osb[:].rearrange("p (f d) -> p f d", f=FUT),
                        in0=ps[:].rearrange("p (f d) -> p f d", f=FUT),
                        scalar=1.0,
                        in1=resb,
                        op0=AL.mult, op1=AL.add)
                    p0 = PAST if c == 0 else 0
                    l0 = c * P - PAST + p0
                    nc.sync.dma_start(o_v[b, l0:l0 + (P - p0), :], osb[p0:P, :])
    nc.finalize()
    return nc


_CACHE = {}


def _get_nc():
    if "nc" not in _CACHE:
        _CACHE["nc"] = build()
    return _CACHE["nc"]


def kernel(inp, W_out, b_out, masks=None, **_unused):
    """Full-input entry point.  Shards batch over 8 cores, runs the Bass
    kernel, reassembles full outputs (o, attn_stack)."""
    from concourse.bass_utils import run_bass_kernel_spmd

    inp = np.ascontiguousarray(np.asarray(inp, dtype=np.float32))
    W_out = np.asarray(W_out, dtype=np.float32)
    b_out = np.asarray(b_out, dtype=np.float32).reshape(1, DIN)

    wt = np.ascontiguousarray(W_out.T)              # [D, j]
    b_hi = b_out.astype(mybir_bf16_np())
    b_lo = (b_out - b_hi.astype(np.float32)).astype(mybir_bf16_np())

    nc = _get_nc()
    in_maps = []
    for core in range(NCORES):
        shard = inp[core * BPC:(core + 1) * BPC]
        in_maps.append({
            "inp": np.ascontiguousarray(shard),
            "wt": wt,
            "bhi": b_hi,
            "blo": b_lo,
        })

    res = run_bass_kernel_spmd(
        nc, in_maps, core_ids=list(range(NCORES)),
        trace=bool(int(os.environ.get("KERNEL_TRACE", "0"))),
    )
    _CACHE["last_result"] = res

    o_full = np.concatenate([r["o"] for r in res.results], axis=0)
    attn_full = np.concatenate([r["attn"] for r in res.results], axis=0)
    return o_full, attn_full


def mybir_bf16_np():
    import ml_dtypes
    return ml_dtypes.bfloat16


# revision 39
# speedup vs baseline: 1.8679x; 1.8679x over previous
"""Trainium2 Bass kernel for nn_Attn_82832739270904 (sparse dilated attention).

Math (per batch, L=1024, D=256):
  3 layers of dilated causal attention with window 2: layer i (d = 1,2,4)
  lets position t attend only to {t, t-d}.  So softmax rows have <=2
  nonzeros; attn output = two diagonals per [L,L] slice, rest exact 0.0.
    s0[t] = out[t].out[t]/16, s1[t] = out[t].out[t-d]/16
    (a0,a1) = softmax(s0,s1);  out' = (1+a0)*out + a1*out[t-d]
  Final: o = out3[:,16:] @ W^T + b, reshaped [.,4,64], + inp[:,16:,:64].

Sharding: pure data parallel, 2 batches per core across 8 cores.

Layout on chip: sequence on partitions (8 chunks of 128), D on free dim.
Shifted-by-d copies made via SBUF->SBUF DMA.  Dots via fused
tensor_tensor_reduce (DVE) / activation-Square-accumulate (ACT).  Apply via
scalar_tensor_tensor + activation-scale.  Final linear on PE after
PE-transposes, bias folded in as K=1 matmuls (bf16 hi/lo split), residual
added with a broadcast AP.  attn written as 24 x 1MiB zero DMAs + strided
diagonal DMAs.
"""

import os
import sys

import numpy as np

for _p in ("/opt/trn_rl_repo", "/root/.axon_site/_ro/trn_rl_repo"):
    if os.path.isdir(_p) and _p not in sys.path:
        sys.path.insert(0, _p)

import concourse.bacc as bacc
import concourse.bass as bass
import concourse.mybir as mybir
from concourse.masks import make_identity
from concourse.tile import TileContext

P = 128
B, L, DIN = 16, 1024, 256
PAST, FUT, DIM = 16, 4, 64
NL = 3
DILS = (1, 2, 4)
NCORES = 8
BPC = B // NCORES            # batches per core
NCH = L // P                 # seq chunks per batch
CW = DIN                     # cols per chunk in the [128, 2048] layout
ROW = NCH * CW
SLICE = L * L                # elements in one [L, L] attn slice
f32 = mybir.dt.float32
f32r = mybir.dt.float32r
bf16 = mybir.dt.bfloat16
AL = mybir.AluOpType
AF = mybir.ActivationFunctionType


def _dview(flat_ap, offset, dims):
    """Arbitrary strided view of a flat DRAM AP (for diagonal writes)."""
    v = flat_ap.copy()
    v.offset = offset
    v.ap = mybir.VecI64Pair(dims)
    return v


def build():
    nc = bacc.Bacc(None, target_bir_lowering=False, debug=False)
    inp = nc.dram_tensor("inp", [BPC, L, DIN], f32, kind="ExternalInput")
    wt = nc.dram_tensor("wt", [DIN, DIN], f32, kind="ExternalInput")  # W_out.T
    bhi = nc.dram_tensor("bhi", [1, DIN], bf16, kind="ExternalInput")
    blo = nc.dram_tensor("blo", [1, DIN], bf16, kind="ExternalInput")
    attn = nc.dram_tensor("attn", [BPC, NL, L, L], f32, kind="ExternalOutput")
    o = nc.dram_tensor("o", [BPC, L - PAST, FUT, DIM], f32, kind="ExternalOutput")

    attn_flat = attn[:].flatten()
    o_v = o[:].rearrange("b l f d -> b l (f d)")

    with TileContext(nc) as tc:
        with (
            tc.tile_pool(name="const", bufs=1) as cp,
            tc.tile_pool(name="outs", bufs=(NL + 1) * BPC) as outp,
            tc.tile_pool(name="shps", bufs=4, space="PSUM") as pshp,
            tc.tile_pool(name="scores", bufs=NL * BPC) as scp,
            tc.tile_pool(name="attw", bufs=NL * BPC) as awp,
            tc.tile_pool(name="soft", bufs=4) as sfp,
            tc.tile_pool(name="coef", bufs=NL * BPC) as cfp,
            tc.tile_pool(name="jdve", bufs=2) as jdp,
            tc.tile_pool(name="jact", bufs=2) as jap,
            tc.tile_pool(name="uu", bufs=3) as upool,
            tc.tile_pool(name="o3t", bufs=4) as tpool,
            tc.tile_pool(name="osb", bufs=3) as osp,
            tc.tile_pool(name="tps", bufs=2, space="PSUM") as tps,
            tc.tile_pool(name="ops", bufs=2, space="PSUM") as opsum,
        ):
            # ---- constants ----
            zeros = cp.tile([P, 2048], f32)
            nc.vector.memset(zeros[:], 0.0)
            ident = cp.tile([P, P], f32)
            make_identity(nc, ident[:])
            ones_bf = cp.tile([1, P], bf16)
            nc.vector.memset(ones_bf[:], 1.0)
            wt_t = []
            for h in range(2):
                t = cp.tile([P, DIN], f32r, name=f"wt{h}", tag=f"wt{h}")
                nc.gpsimd.dma_start(t[:], wt[h * P:(h + 1) * P, :])
                wt_t.append(t)
            bhi_t = cp.tile([1, DIN], bf16)
            nc.sync.dma_start(bhi_t[:], bhi[:])
            blo_t = cp.tile([1, DIN], bf16)
            nc.sync.dma_start(blo_t[:], blo[:])

            # shift matrices: SM1[l][k, m] = 1 iff k == m - d (within-chunk),
            # SM2[l][k, m] = 1 iff k == 128 + m - d (from previous chunk)
            sm1, sm2 = [], []
            for l, d in enumerate(DILS):
                for dst, base in ((sm1, d), (sm2, d - P)):
                    m = cp.tile([P, P], f32, name=f"sm_{l}_{base}", tag=f"sm_{l}_{base}")
                    nc.gpsimd.memset(m[:], 0.0)
                    nc.gpsimd.affine_select(
                        out=m[:], in_=m[:], compare_op=AL.not_equal, fill=1.0,
                        base=base, pattern=[[-1, P]], channel_multiplier=1)
                    dst.append(m)

            # ---- load inp into layer-0 out tiles ----
            out_t = [[outp.tile([P, ROW], f32, name=f"out_l{l}_b{b}", tag="out")
                      for b in range(BPC)] for l in range(NL + 1)]
            for b in range(BPC):
                nc.sync.dma_start(
                    out_t[0][b][:].rearrange("p (c d) -> p c d", c=NCH),
                    inp[b, :, :].rearrange("(c p) d -> p c d", p=P),
                )

            # ---- zero-fill attn (24 x 1MiB) on SWDGE: HWDGE traffic starves
            # SWDGE queues, so the bulk stream must be the SWDGE one ----
            for l in range(NL):
                for b in range(BPC):
                    base = (b * NL + l) * SLICE
                    for q in range(4):
                        dst = _dview(attn_flat, base + q * (SLICE // 4),
                                     [[2048, P], [1, 2048]])
                        nc.gpsimd.dma_start(dst, zeros[:])

            # ---- attention layers ----
            deferred_diags = []

            def emit_diags(l, b, d, a_t):
                base = (b * NL + l) * SLICE
                nc.gpsimd.dma_start(
                    _dview(attn_flat, base, [[1025, P], [P * 1025, NCH]]),
                    a_t[:, 0:NCH])
                nc.gpsimd.dma_start(
                    _dview(attn_flat, base + 1024 * d,
                           [[1025, P - d], [1, 1]]),
                    a_t[d:P, NCH:NCH + 1])
                nc.gpsimd.dma_start(
                    _dview(attn_flat, base + P * 1025 - d,
                           [[1025, P], [P * 1025, NCH - 1]]),
                    a_t[:, NCH + 1:2 * NCH])

            for l in range(NL):
                d = DILS[l]
                cur = out_t[l]
                nxt = out_t[l + 1]
                # previous layer's diagonal writes go out now: their zero-fill
                # and coefficients are both ready, so they don't stall the ring
                for args in deferred_diags:
                    emit_diags(*args)
                deferred_diags = []

                for b in range(BPC):
                    # per-batch coefficient tiles so the two batch pipelines
                    # never serialize on shared softmax state
                    s_t = scp.tile([P, 16], f32, name=f"s_{l}_{b}", tag="s_t")
                    a_t = awp.tile([P, 16], f32, name=f"a_{l}_{b}", tag="a_t")
                    m_t = sfp.tile([P, NCH], f32, name=f"m_{l}_{b}", tag="m_t")
                    z_t = sfp.tile([P, NCH], f32, name=f"z_{l}_{b}", tag="z_t")
                    r_t = sfp.tile([P, NCH], f32, name=f"r_{l}_{b}", tag="r_t")
                    p1_t = cfp.tile([P, NCH], f32, name=f"p1_{l}_{b}", tag="p1_t")
                    rg_t = sfp.tile([P, NCH], f32, name=f"rg_{l}_{b}", tag="rg_t")
                    g_t = cfp.tile([P, NCH], f32, name=f"g_{l}_{b}", tag="g_t")

                    # shift via PE in double-chunks (exact 0/1 permutation
                    # matmuls into one PSUM bank each); s0 on ACT
                    # (square+accum); s1 fused STT on DVE reading PSUM
                    psh = []
                    for dc in range(NCH // 2):
                        ds = dc * 2 * CW
                        sp = pshp.tile([P, 2 * CW], f32,
                                       name=f"psh_{l}_{b}_{dc}", tag="psh")
                        nc.tensor.matmul(sp[:], sm1[l][:],
                                         cur[b][:, ds:ds + 2 * CW],
                                         start=True, stop=(dc == 0))
                        if dc == 0:
                            nc.tensor.matmul(sp[:, CW:2 * CW], sm2[l][:],
                                             cur[b][:, 0:CW],
                                             start=False, stop=True)
                        else:
                            nc.tensor.matmul(sp[:], sm2[l][:],
                                             cur[b][:, ds - CW:ds + CW],
                                             start=False, stop=True)
                        psh.append(sp)
                        for h in range(2):
                            c = 2 * dc + h
                            cs = c * CW
                            ja = jap.tile([P, CW], f32)
                            nc.scalar.activation(
                                ja[:], cur[b][:, cs:cs + CW], AF.Square,
                                scale=0.25, accum_out=s_t[:, c:c + 1])
                            jv = jdp.tile([P, CW], f32)
                            nc.vector.scalar_tensor_tensor(
                                out=jv[:],
                                in0=cur[b][:, cs:cs + CW],
                                scalar=0.0625,
                                in1=sp[:, h * CW:(h + 1) * CW],
                                op0=AL.mult, op1=AL.mult,
                                accum_out=s_t[:, NCH + c:NCH + c + 1])

                    # rows t < d attend only to themselves: force s1 = -1e9
                    nc.vector.memset(s_t[0:d, NCH:NCH + 1], -1e9)

                    # pairwise softmax over (s0, s1) for this batch
                    nc.vector.tensor_tensor(m_t[:], s_t[:, 0:NCH],
                                            s_t[:, NCH:2 * NCH], AL.max)
                    sv = s_t[:].rearrange("p (x c) -> p x c", x=2)
                    mb = m_t[:].unsqueeze(1).broadcast_to([P, 2, NCH])
                    nc.vector.tensor_tensor(sv, sv, mb, AL.subtract)
                    nc.scalar.activation(a_t[:], s_t[:], AF.Exp)
                    nc.vector.tensor_tensor(z_t[:], a_t[:, 0:NCH],
                                            a_t[:, NCH:2 * NCH], AL.add)
                    nc.vector.reciprocal(r_t[:], z_t[:])
                    av = a_t[:].rearrange("p (x c) -> p x c", x=2)
                    rb = r_t[:].unsqueeze(1).broadcast_to([P, 2, NCH])
                    nc.vector.tensor_tensor(av, av, rb, AL.mult)
                    nc.vector.tensor_scalar_add(p1_t[:], a_t[:, 0:NCH], 1.0)
                    nc.vector.reciprocal(rg_t[:], p1_t[:])
                    nc.vector.tensor_tensor(g_t[:], a_t[:, NCH:2 * NCH],
                                            rg_t[:], AL.mult)

                    # diagonal writes deferred until the next layer's shifts
                    deferred_diags.append((l, b, d, a_t))

                    # apply: out' = (1+a0) * (out + g * shifted)
                    for c in range(NCH):
                        cs = c * CW
                        u = upool.tile([P, CW], f32)
                        nc.vector.scalar_tensor_tensor(
                            out=u[:],
                            in0=psh[c // 2][:, (c % 2) * CW:(c % 2 + 1) * CW],
                            scalar=g_t[:, c:c + 1],
                            in1=cur[b][:, cs:cs + CW],
                            op0=AL.mult, op1=AL.add)
                        if c % 2 == 0:
                            nc.scalar.activation(
                                nxt[b][:, cs:cs + CW], u[:], AF.Copy,
                                scale=p1_t[:, c:c + 1])
                        else:
                            nc.vector.tensor_scalar_mul(
                                nxt[b][:, cs:cs + CW], u[:], p1_t[:, c:c + 1])

            # last layer's diagonal writes
            for args in deferred_diags:
                emit_diags(*args)
            deferred_diags = []

            # ---- final linear + bias + residual ----
            fin = out_t[NL]
            for b in range(BPC):
                o3 = [tpool.tile([P, NCH * P], f32r, name=f"o3_b{b}_h{h}", tag="o3")
                      for h in range(2)]
                for c in range(NCH):
                    for h in range(2):
                        ps = tps.tile([P, P], f32)
                        nc.tensor.transpose(
                            ps[:], fin[b][:, c * CW + h * P:c * CW + (h + 1) * P],
                            ident[:])
                        nc.scalar.copy(o3[h][:, c * P:(c + 1) * P], ps[:])
                for c in range(NCH):
                    ps = opsum.tile([P, DIN], f32)
                    nc.tensor.matmul(ps[:], o3[0][:, c * P:(c + 1) * P], wt_t[0][:],
                                     start=True, stop=False)
                    nc.tensor.matmul(ps[:], o3[1][:, c * P:(c + 1) * P], wt_t[1][:],
                                     start=False, stop=False)
                    nc.tensor.matmul(ps[:], ones_bf[:], bhi_t[:],
                                     start=False, stop=False)
                    nc.tensor.matmul(ps[:], ones_bf[:], blo_t[:],
                                     start=False, stop=True)
                    osb = osp.tile([P, DIN], f32)
                    res = out_t[0][b][:, c * CW:c * CW + DIM]
                    resb = res.unsqueeze(1).broadcast_to([P, FUT, DIM])
                    nc.vector.scalar_tensor_tensor(
                        out=osb[:].rearrange("p (f d) -> p f d", f=FUT),
                        in0=ps[:].rearrange("p (f d) -> p f d", f=FUT),
                        scalar=1.0,
                        in1=resb,
                        op0=AL.mult, op1=AL.add)
                    p0 = PAST if c == 0 else 0
                    l0 = c * P - PAST + p0
                    nc.sync.dma_start(o_v[b, l0:l0 + (P - p0), :], osb[p0:P, :])
    nc.finalize()
    return nc


_CACHE = {}


def _get_nc():
    if "nc" not in _CACHE:
        _CACHE["nc"] = build()
    return _CACHE["nc"]


def kernel(inp, W_out, b_out, masks=None, **_unused):
    """Full-input entry point.  Shards batch over 8 cores, runs the Bass
    kernel, reassembles full outputs (o, attn_stack)."""
    from concourse.bass_utils import run_bass_kernel_spmd

    inp = np.ascontiguousarray(np.asarray(inp, dtype=np.float32))
    W_out = np.asarray(W_out, dtype=np.float32)
    b_out = np.asarray(b_out, dtype=np.float32).reshape(1, DIN)

    wt = np.ascontiguousarray(W_out.T)              # [D, j]
    b_hi = b_out.astype(mybir_bf16_np())
    b_lo = (b_out - b_hi.astype(np.float32)).astype(mybir_bf16_np())

    nc = _get_nc()
    in_maps = []
    for core in range(NCORES):
        shard = inp[core * BPC:(core + 1) * BPC]
        in_maps.append({
            "inp": np.ascontiguousarray(shard),
            "wt": wt,
            "bhi": b_hi,
            "blo": b_lo,
        })

    res = run_bass_kernel_spmd(
        nc, in_maps, core_ids=list(range(NCORES)),
        trace=bool(int(os.environ.get("KERNEL_TRACE", "0"))),
    )
    _CACHE["last_result"] = res

    o_full = np.concatenate([r["o"] for r in res.results], axis=0)
    attn_full = np.concatenate([r["attn"] for r in res.results], axis=0)
    return o_full, attn_full


def mybir_bf16_np():
    import ml_dtypes
    return ml_dtypes.bfloat16
